# revision 13
# baseline (speedup 1.0000x reference)
"""AddAttention kernel for Trainium2, SPMD across 8 NeuronCores.

Math: score[b,i,j] = sq[b,i] + sk[b,j] with sq = inp@wq, sk = inp@wk.
softmax over j cancels the sq[b,i] term exactly, so
    attn[b,i,:] = softmax(sk[b,:])   (identical for every i)
    out[b,i,:]  = softmax(sk[b,:]) @ inp[b]   (identical for every i)
The kernel computes the 1-D softmax per batch and broadcast-writes the
replicated rows (stride-0 source DMAs). Data-parallel over batch:
2 batches per core. The 16 MB/batch attn writes go on the SP HWDGE
ring; input loads and out writes go on the ACT ring so they are not
FIFO-queued behind them; small SBUF->SBUF shuffles go on SWDGE.
"""

import numpy as np

import concourse.bacc as bacc
import concourse.mybir as mybir
from concourse.tile import TileContext
from concourse.bass_utils import run_bass_kernel_spmd

B, L, D = 16, 2048, 256
NCORES = 8
BPC = B // NCORES  # batches per core
P = 128
NCH = L // P  # 16 rows per partition (p-major layout)

F32 = mybir.dt.float32


def build_bass(repeat=1):
    nc = bacc.Bacc()
    inp = nc.declare_dram_parameter("inp", [BPC, L, D], F32, isOutput=False)
    v_w = nc.declare_dram_parameter("v_w", [1, 2 * D], F32, isOutput=False)
    out = nc.declare_dram_parameter("out", [BPC, L, D], F32, isOutput=True)
    attn = nc.declare_dram_parameter("attn", [BPC, L, L], F32, isOutput=True)

    with TileContext(nc) as tc:
        with (
            tc.tile_pool(name="consts", bufs=1) as cpool,
            tc.tile_pool(name="io", bufs=3) as iopool,
            tc.tile_pool(name="small", bufs=2) as spool,
            tc.tile_pool(name="psum_small", bufs=4, space="PSUM") as psmall,
        ):
            # constants: wk broadcast to all partitions, replicated x16
            wk_b = cpool.tile([P, D], F32)
            nc.sync.dma_start(
                out=wk_b[:], in_=v_w[0:1, D : 2 * D].broadcast_to([P, D])
            )
            wk_rep = cpool.tile([P, NCH, D], F32)
            nc.gpsimd.dma_start(
                out=wk_rep[:],
                in_=wk_b[:].unsqueeze(1).broadcast_to([P, NCH, D]),
            )
            ones_row = cpool.tile([1, P], F32)
            nc.vector.memset(ones_row[:], 1.0)
            ones_col = cpool.tile([P, 1], F32)
            nc.vector.memset(ones_col[:], 1.0)

            for b in [b for _ in range(repeat) for b in range(BPC)]:
                # ---- load inp[b], p-major: partition p holds rows
                # p*16 .. p*16+15  ->  one DMA, 128 x 16 KB descriptors
                inp_sb = iopool.tile([P, NCH, D], F32, tag="inp_sb")
                nc.scalar.dma_start(
                    out=inp_sb[:],
                    in_=inp[b].rearrange("(p c) d -> p c d", c=NCH),
                )

                # ---- sk[p, c] = <row p*16+c, wk>  (one mult + one reduce)
                prod = spool.tile([P, NCH, D], F32, tag="prod")
                sk = spool.tile([P, NCH], F32, tag="sk")
                nc.vector.tensor_tensor(
                    out=prod[:],
                    in0=inp_sb[:],
                    in1=wk_rep[:],
                    op=mybir.AluOpType.mult,
                )
                nc.vector.tensor_reduce(
                    out=sk[:],
                    in_=prod[:],
                    axis=mybir.AxisListType.X,
                    op=mybir.AluOpType.add,
                )

                # ---- e = exp(sk); |sk| < ~5 so no max-subtraction needed.
                # accum_out gives per-partition row sums for free.
                e_wide = spool.tile([P, NCH], F32, tag="e_wide")
                rowsum = spool.tile([P, 1], F32, tag="rowsum")
                nc.scalar.activation(
                    e_wide[:],
                    sk[:],
                    mybir.ActivationFunctionType.Exp,
                    accum_out=rowsum[:],
                )

                # ---- S = sum over partitions (ones matmul), rinv = 1/S
                psum_S = psmall.tile([1, 1], F32, tag="ps")
                nc.tensor.matmul(
                    psum_S[:], lhsT=ones_col[:], rhs=rowsum[:], start=True, stop=True
                )
                rinv = spool.tile([1, 1], F32, tag="rinv")
                nc.vector.reciprocal(rinv[:], psum_S[:])
                rinv_row = spool.tile([1, P], F32, tag="rinv_row")
                nc.vector.tensor_scalar_mul(rinv_row[:], ones_row[:], rinv[:])

                # ---- e_row[0, l] = e_wide[p, c], l = p*16+c (p-major gather)
                e_row = spool.tile([1, L], F32, tag="e_row")
                nc.gpsimd.dma_start(
                    out=e_row[0:1, :].rearrange("a (p c) -> a p c", p=P),
                    in_=e_wide[:],
                )

                # ---- attn rows: [128, 2048] tile, every row = rinv * e_row
                attn_sb = iopool.tile([P, L], F32, tag="attn_sb")
                NB = 512  # matmul moving free dim / PSUM bank
                for n in range(L // NB):
                    psum_attn = psmall.tile([P, NB], F32, tag="ps")
                    nc.tensor.matmul(
                        psum_attn[:],
                        lhsT=rinv_row[:],
                        rhs=e_row[0:1, n * NB : (n + 1) * NB],
                        start=True,
                        stop=True,
                    )
                    nc.scalar.copy(attn_sb[:, n * NB : (n + 1) * NB], psum_attn[:])

                # all attn rows are identical -> stride-0 source broadcast DMA
                nc.sync.dma_start(
                    out=attn[b].rearrange("(p c) j -> p c j", c=NCH),
                    in_=attn_sb[:].unsqueeze(1).broadcast_to([P, NCH, L]),
                )

                # ---- row_out = e @ inp[b]  (K=128 per chunk, accumulate)
                psum_ro = psmall.tile([1, D], F32, tag="ps")
                for c in range(NCH):
                    nc.tensor.matmul(
                        psum_ro[:],
                        lhsT=e_wide[:, c : c + 1],
                        rhs=inp_sb[:, c, :],
                        start=(c == 0),
                        stop=(c == NCH - 1),
                    )
                ro_row = spool.tile([1, D], F32, tag="ro_row")
                nc.vector.tensor_copy(ro_row[:], psum_ro[:])

                # ---- out rows: [128, 256] = rinv * row_out, replicated x16
                # per partition so the HBM write uses 16 KB descriptors
                psum_obc = psmall.tile([P, D], F32, tag="ps")
                nc.tensor.matmul(
                    psum_obc[:], lhsT=rinv_row[:], rhs=ro_row[:], start=True, stop=True
                )
                out_sb = spool.tile([P, D], F32, tag="out_sb")
                nc.vector.tensor_copy(out_sb[:], psum_obc[:])
                out_rep = spool.tile([P, NCH, D], F32, tag="out_rep")
                nc.gpsimd.dma_start(
                    out=out_rep[:],
                    in_=out_sb[:].unsqueeze(1).broadcast_to([P, NCH, D]),
                )
                nc.scalar.dma_start(
                    out=out[b].rearrange("(p c) d -> p c d", c=NCH),
                    in_=out_rep[:],
                )
    return nc


_NC_CACHE = None


def _get_nc():
    global _NC_CACHE
    if _NC_CACHE is None:
        nc = build_bass()
        if not nc.is_finalized():
            nc.finalize()
        _NC_CACHE = nc
    return _NC_CACHE


def kernel(inp, v_w, _trace=False, _result_box=None):
    inp = np.ascontiguousarray(inp, dtype=np.float32)
    v_w = np.ascontiguousarray(v_w, dtype=np.float32)
    nc = _get_nc()
    in_maps = [
        {"inp": inp[i * BPC : (i + 1) * BPC], "v_w": v_w} for i in range(NCORES)
    ]
    res = run_bass_kernel_spmd(nc, in_maps, core_ids=list(range(NCORES)), trace=_trace)
    if _result_box is not None:
        _result_box.append(res)
    out = np.concatenate([res.results[i]["out"] for i in range(NCORES)], axis=0)
    attn = np.concatenate([res.results[i]["attn"] for i in range(NCORES)], axis=0)
    return out, attn


# revision 15
# speedup vs baseline: 1.0190x; 1.0190x over previous
"""AddAttention kernel for Trainium2, SPMD across 8 NeuronCores.

Math: score[b,i,j] = sq[b,i] + sk[b,j] with sq = inp@wq, sk = inp@wk.
softmax over j cancels the sq[b,i] term exactly, so
    attn[b,i,:] = softmax(sk[b,:])   (identical for every i)
    out[b,i,:]  = softmax(sk[b,:]) @ inp[b]   (identical for every i)
The kernel computes the 1-D softmax per batch and broadcast-writes the
replicated rows (stride-0 source DMAs). Data-parallel over batch:
2 batches per core. The 16 MB/batch attn writes go on the SP HWDGE
ring; input loads and out writes go on the ACT ring so they are not
FIFO-queued behind them; small SBUF->SBUF shuffles go on SWDGE.
"""

import numpy as np

import concourse.bacc as bacc
import concourse.mybir as mybir
from concourse.tile import TileContext
from concourse.bass_utils import run_bass_kernel_spmd

B, L, D = 16, 2048, 256
NCORES = 8
BPC = B // NCORES  # batches per core
P = 128
NCH = L // P  # 16 rows per partition (p-major layout)

F32 = mybir.dt.float32


def build_bass(repeat=1):
    nc = bacc.Bacc()
    inp = nc.declare_dram_parameter("inp", [BPC, L, D], F32, isOutput=False)
    v_w = nc.declare_dram_parameter("v_w", [1, 2 * D], F32, isOutput=False)
    out = nc.declare_dram_parameter("out", [BPC, L, D], F32, isOutput=True)
    attn = nc.declare_dram_parameter("attn", [BPC, L, L], F32, isOutput=True)

    with TileContext(nc) as tc:
        with (
            tc.tile_pool(name="consts", bufs=1) as cpool,
            tc.tile_pool(name="io", bufs=3) as iopool,
            tc.tile_pool(name="small", bufs=2) as spool,
            tc.tile_pool(name="psum_small", bufs=4, space="PSUM") as psmall,
        ):
            # constants: wk broadcast to all partitions, replicated x16
            wk_b = cpool.tile([P, D], F32)
            nc.sync.dma_start(
                out=wk_b[:], in_=v_w[0:1, D : 2 * D].broadcast_to([P, D])
            )
            wk_rep = cpool.tile([P, NCH, D], F32)
            nc.gpsimd.dma_start(
                out=wk_rep[:],
                in_=wk_b[:].unsqueeze(1).broadcast_to([P, NCH, D]),
            )
            ones_row = cpool.tile([1, P], F32)
            nc.vector.memset(ones_row[:], 1.0)
            ones_col = cpool.tile([P, 1], F32)
            nc.vector.memset(ones_col[:], 1.0)

            for b in [b for _ in range(repeat) for b in range(BPC)]:
                # ---- load inp[b], p-major: partition p holds rows
                # p*16 .. p*16+15  ->  one DMA, 128 x 16 KB descriptors
                inp_sb = iopool.tile([P, NCH, D], F32, tag="inp_sb")
                nc.scalar.dma_start(
                    out=inp_sb[:],
                    in_=inp[b].rearrange("(p c) d -> p c d", c=NCH),
                )

                # ---- sk[p, c] = <row p*16+c, wk>  (one mult + one reduce)
                prod = spool.tile([P, NCH, D], F32, tag="prod")
                sk = spool.tile([P, NCH], F32, tag="sk")
                nc.vector.tensor_tensor(
                    out=prod[:],
                    in0=inp_sb[:],
                    in1=wk_rep[:],
                    op=mybir.AluOpType.mult,
                )
                nc.vector.tensor_reduce(
                    out=sk[:],
                    in_=prod[:],
                    axis=mybir.AxisListType.X,
                    op=mybir.AluOpType.add,
                )

                # ---- e = exp(sk); |sk| < ~5 so no max-subtraction needed.
                # accum_out gives per-partition row sums for free.
                e_wide = spool.tile([P, NCH], F32, tag="e_wide")
                rowsum = spool.tile([P, 1], F32, tag="rowsum")
                nc.scalar.activation(
                    e_wide[:],
                    sk[:],
                    mybir.ActivationFunctionType.Exp,
                    accum_out=rowsum[:],
                )

                # ---- S = sum over partitions (ones matmul), rinv = 1/S
                psum_S = psmall.tile([1, 1], F32, tag="ps")
                nc.tensor.matmul(
                    psum_S[:], lhsT=ones_col[:], rhs=rowsum[:], start=True, stop=True
                )
                rinv = spool.tile([1, 1], F32, tag="rinv")
                nc.vector.reciprocal(rinv[:], psum_S[:])
                rinv_row = spool.tile([1, P], F32, tag="rinv_row")
                nc.vector.tensor_scalar_mul(rinv_row[:], ones_row[:], rinv[:])

                # ---- e_row[0, l] = e_wide[p, c], l = p*16+c (p-major gather)
                e_row = spool.tile([1, L], F32, tag="e_row")
                nc.gpsimd.dma_start(
                    out=e_row[0:1, :].rearrange("a (p c) -> a p c", p=P),
                    in_=e_wide[:],
                )

                # ---- attn rows: [128, 2048] tile, every row = rinv * e_row
                attn_sb = iopool.tile([P, L], F32, tag="attn_sb")
                NB = 512  # matmul moving free dim / PSUM bank
                for n in range(L // NB):
                    psum_attn = psmall.tile([P, NB], F32, tag="ps")
                    nc.tensor.matmul(
                        psum_attn[:],
                        lhsT=rinv_row[:],
                        rhs=e_row[0:1, n * NB : (n + 1) * NB],
                        start=True,
                        stop=True,
                    )
                    nc.scalar.copy(attn_sb[:, n * NB : (n + 1) * NB], psum_attn[:])

                # all attn rows are identical -> stride-0 source broadcast
                # DMAs, split across both HWDGE rings (SP + ACT)
                H = NCH // 2
                attn_v = attn[b].rearrange("(p c) j -> p c j", c=NCH)
                nc.sync.dma_start(
                    out=attn_v[:, 0:H, :],
                    in_=attn_sb[:].unsqueeze(1).broadcast_to([P, H, L]),
                )
                nc.scalar.dma_start(
                    out=attn_v[:, H:NCH, :],
                    in_=attn_sb[:].unsqueeze(1).broadcast_to([P, H, L]),
                )

                # ---- row_out = e @ inp[b]  (K=128 per chunk, accumulate)
                psum_ro = psmall.tile([1, D], F32, tag="ps")
                for c in range(NCH):
                    nc.tensor.matmul(
                        psum_ro[:],
                        lhsT=e_wide[:, c : c + 1],
                        rhs=inp_sb[:, c, :],
                        start=(c == 0),
                        stop=(c == NCH - 1),
                    )
                ro_row = spool.tile([1, D], F32, tag="ro_row")
                nc.vector.tensor_copy(ro_row[:], psum_ro[:])

                # ---- out rows: [128, 256] = rinv * row_out, replicated x16
                # per partition so the HBM write uses 16 KB descriptors
                psum_obc = psmall.tile([P, D], F32, tag="ps")
                nc.tensor.matmul(
                    psum_obc[:], lhsT=rinv_row[:], rhs=ro_row[:], start=True, stop=True
                )
                out_sb = spool.tile([P, D], F32, tag="out_sb")
                nc.vector.tensor_copy(out_sb[:], psum_obc[:])
                out_rep = spool.tile([P, NCH, D], F32, tag="out_rep")
                nc.vector.tensor_copy(
                    out_rep[:], out_sb[:].unsqueeze(1).broadcast_to([P, NCH, D])
                )
                nc.sync.dma_start(
                    out=out[b].rearrange("(p c) d -> p c d", c=NCH),
                    in_=out_rep[:],
                )
    return nc


_NC_CACHE = None


def _get_nc():
    global _NC_CACHE
    if _NC_CACHE is None:
        nc = build_bass()
        if not nc.is_finalized():
            nc.finalize()
        _NC_CACHE = nc
    return _NC_CACHE


def kernel(inp, v_w, _trace=False, _result_box=None):
    inp = np.ascontiguousarray(inp, dtype=np.float32)
    v_w = np.ascontiguousarray(v_w, dtype=np.float32)
    nc = _get_nc()
    in_maps = [
        {"inp": inp[i * BPC : (i + 1) * BPC], "v_w": v_w} for i in range(NCORES)
    ]
    res = run_bass_kernel_spmd(nc, in_maps, core_ids=list(range(NCORES)), trace=_trace)
    if _result_box is not None:
        _result_box.append(res)
    out = np.concatenate([res.results[i]["out"] for i in range(NCORES)], axis=0)
    attn = np.concatenate([res.results[i]["attn"] for i in range(NCORES)], axis=0)
    return out, attn


# revision 16
# speedup vs baseline: 1.0230x; 1.0039x over previous
"""AddAttention kernel for Trainium2, SPMD across 8 NeuronCores.

Math: score[b,i,j] = sq[b,i] + sk[b,j] with sq = inp@wq, sk = inp@wk.
softmax over j cancels the sq[b,i] term exactly, so
    attn[b,i,:] = softmax(sk[b,:])   (identical for every i)
    out[b,i,:]  = softmax(sk[b,:]) @ inp[b]   (identical for every i)
The kernel computes the 1-D softmax per batch and broadcast-writes the
replicated rows (stride-0 source DMAs). Data-parallel over batch:
2 batches per core. The 16 MB/batch attn writes go on the SP HWDGE
ring; input loads and out writes go on the ACT ring so they are not
FIFO-queued behind them; small SBUF->SBUF shuffles go on SWDGE.
"""

import numpy as np

import concourse.bacc as bacc
import concourse.mybir as mybir
from concourse.tile import TileContext
from concourse.bass_utils import run_bass_kernel_spmd

B, L, D = 16, 2048, 256
NCORES = 8
BPC = B // NCORES  # batches per core
P = 128
NCH = L // P  # 16 rows per partition (p-major layout)

F32 = mybir.dt.float32


def build_bass(repeat=1):
    nc = bacc.Bacc()
    inp = nc.declare_dram_parameter("inp", [BPC, L, D], F32, isOutput=False)
    v_w = nc.declare_dram_parameter("v_w", [1, 2 * D], F32, isOutput=False)
    out = nc.declare_dram_parameter("out", [BPC, L, D], F32, isOutput=True)
    attn = nc.declare_dram_parameter("attn", [BPC, L, L], F32, isOutput=True)

    with TileContext(nc) as tc:
        with (
            tc.tile_pool(name="consts", bufs=1) as cpool,
            tc.tile_pool(name="io", bufs=3) as iopool,
            tc.tile_pool(name="small", bufs=2) as spool,
            tc.tile_pool(name="psum_small", bufs=4, space="PSUM") as psmall,
        ):
            # constants: wk broadcast to all partitions, replicated x16
            wk_b = cpool.tile([P, D], F32)
            nc.sync.dma_start(
                out=wk_b[:], in_=v_w[0:1, D : 2 * D].broadcast_to([P, D])
            )
            wk_rep = cpool.tile([P, NCH, D], F32)
            nc.gpsimd.dma_start(
                out=wk_rep[:],
                in_=wk_b[:].unsqueeze(1).broadcast_to([P, NCH, D]),
            )
            ones_row = cpool.tile([1, P], F32)
            nc.vector.memset(ones_row[:], 1.0)
            ones_col = cpool.tile([P, 1], F32)
            nc.vector.memset(ones_col[:], 1.0)

            for b in [b for _ in range(repeat) for b in range(BPC)]:
                # ---- load inp[b], p-major: partition p holds rows
                # p*16 .. p*16+15. Quartered so sk compute pipelines with
                # the load instead of waiting for the full 2 MB.
                inp_sb = iopool.tile([P, NCH, D], F32, tag="inp_sb")
                inp_v = inp[b].rearrange("(p c) d -> p c d", c=NCH)
                QL = NCH // 4
                for q in range(4):
                    nc.scalar.dma_start(
                        out=inp_sb[:, q * QL : (q + 1) * QL, :],
                        in_=inp_v[:, q * QL : (q + 1) * QL, :],
                    )

                # ---- sk[p, c] = <row p*16+c, wk>  (mult + reduce per quarter)
                prod = spool.tile([P, NCH, D], F32, tag="prod")
                sk = spool.tile([P, NCH], F32, tag="sk")
                for q in range(4):
                    qs = slice(q * QL, (q + 1) * QL)
                    nc.vector.tensor_tensor(
                        out=prod[:, qs, :],
                        in0=inp_sb[:, qs, :],
                        in1=wk_rep[:, qs, :],
                        op=mybir.AluOpType.mult,
                    )
                    nc.vector.tensor_reduce(
                        out=sk[:, qs],
                        in_=prod[:, qs, :],
                        axis=mybir.AxisListType.X,
                        op=mybir.AluOpType.add,
                    )

                # ---- e = exp(sk); |sk| < ~5 so no max-subtraction needed.
                # accum_out gives per-partition row sums for free.
                e_wide = spool.tile([P, NCH], F32, tag="e_wide")
                rowsum = spool.tile([P, 1], F32, tag="rowsum")
                nc.scalar.activation(
                    e_wide[:],
                    sk[:],
                    mybir.ActivationFunctionType.Exp,
                    accum_out=rowsum[:],
                )

                # ---- S = sum over partitions (ones matmul), rinv = 1/S
                psum_S = psmall.tile([1, 1], F32, tag="ps")
                nc.tensor.matmul(
                    psum_S[:], lhsT=ones_col[:], rhs=rowsum[:], start=True, stop=True
                )
                rinv = spool.tile([1, 1], F32, tag="rinv")
                nc.vector.reciprocal(rinv[:], psum_S[:])
                rinv_row = spool.tile([1, P], F32, tag="rinv_row")
                nc.vector.tensor_scalar_mul(rinv_row[:], ones_row[:], rinv[:])

                # ---- e_row[0, l] = e_wide[p, c], l = p*16+c (p-major gather)
                e_row = spool.tile([1, L], F32, tag="e_row")
                nc.gpsimd.dma_start(
                    out=e_row[0:1, :].rearrange("a (p c) -> a p c", p=P),
                    in_=e_wide[:],
                )

                # ---- attn rows: [128, 2048] tile, every row = rinv * e_row
                attn_sb = iopool.tile([P, L], F32, tag="attn_sb")
                NB = 512  # matmul moving free dim / PSUM bank
                for n in range(L // NB):
                    psum_attn = psmall.tile([P, NB], F32, tag="ps")
                    nc.tensor.matmul(
                        psum_attn[:],
                        lhsT=rinv_row[:],
                        rhs=e_row[0:1, n * NB : (n + 1) * NB],
                        start=True,
                        stop=True,
                    )
                    nc.scalar.copy(attn_sb[:, n * NB : (n + 1) * NB], psum_attn[:])

                # all attn rows are identical -> stride-0 source broadcast
                # DMAs, split across both HWDGE rings (SP + ACT)
                H = NCH // 2
                attn_v = attn[b].rearrange("(p c) j -> p c j", c=NCH)
                nc.sync.dma_start(
                    out=attn_v[:, 0:H, :],
                    in_=attn_sb[:].unsqueeze(1).broadcast_to([P, H, L]),
                )
                nc.scalar.dma_start(
                    out=attn_v[:, H:NCH, :],
                    in_=attn_sb[:].unsqueeze(1).broadcast_to([P, H, L]),
                )

                # ---- row_out = e @ inp[b]  (K=128 per chunk, accumulate)
                psum_ro = psmall.tile([1, D], F32, tag="ps")
                for c in range(NCH):
                    nc.tensor.matmul(
                        psum_ro[:],
                        lhsT=e_wide[:, c : c + 1],
                        rhs=inp_sb[:, c, :],
                        start=(c == 0),
                        stop=(c == NCH - 1),
                    )
                ro_row = spool.tile([1, D], F32, tag="ro_row")
                nc.vector.tensor_copy(ro_row[:], psum_ro[:])

                # ---- out rows: [128, 256] = rinv * row_out, replicated x16
                # per partition so the HBM write uses 16 KB descriptors
                psum_obc = psmall.tile([P, D], F32, tag="ps")
                nc.tensor.matmul(
                    psum_obc[:], lhsT=rinv_row[:], rhs=ro_row[:], start=True, stop=True
                )
                out_sb = spool.tile([P, D], F32, tag="out_sb")
                nc.vector.tensor_copy(out_sb[:], psum_obc[:])
                out_rep = spool.tile([P, NCH, D], F32, tag="out_rep")
                nc.vector.tensor_copy(
                    out_rep[:], out_sb[:].unsqueeze(1).broadcast_to([P, NCH, D])
                )
                nc.sync.dma_start(
                    out=out[b].rearrange("(p c) d -> p c d", c=NCH),
                    in_=out_rep[:],
                )
    return nc


_NC_CACHE = None


def _get_nc():
    global _NC_CACHE
    if _NC_CACHE is None:
        nc = build_bass()
        if not nc.is_finalized():
            nc.finalize()
        _NC_CACHE = nc
    return _NC_CACHE


def kernel(inp, v_w, _trace=False, _result_box=None):
    inp = np.ascontiguousarray(inp, dtype=np.float32)
    v_w = np.ascontiguousarray(v_w, dtype=np.float32)
    nc = _get_nc()
    in_maps = [
        {"inp": inp[i * BPC : (i + 1) * BPC], "v_w": v_w} for i in range(NCORES)
    ]
    res = run_bass_kernel_spmd(nc, in_maps, core_ids=list(range(NCORES)), trace=_trace)
    if _result_box is not None:
        _result_box.append(res)
    out = np.concatenate([res.results[i]["out"] for i in range(NCORES)], axis=0)
    attn = np.concatenate([res.results[i]["attn"] for i in range(NCORES)], axis=0)
    return out, attn
